# revision 33
# baseline (speedup 1.0000x reference)
"""CEMSA (conv-embedded multi-head spatial-reduction attention) on 8 trn2 cores.

Sharding: core = (batch b, head-half hh).  Each core runs the full SR/KV
path for its batch plus the Q path and attention for its 4 heads, and
produces the partial projection y_part = OT_half.T @ proj_w[:, half].T
(full 4096x256).  Host unshard: out[b] = y_part[2b] + y_part[2b+1] + proj_b.

All convs are executed as per-tap diagonal (depthwise) / dense (pointwise)
matmuls on the PE, reading from a zero-padded [c, 68, 68] image buffer.
Attention is computed transposed (S^T = k^T-tiles @ q^T) so softmax sums
come from a ones-column appended to V; normalization uses a PE broadcast
of per-(head, n) reciprocals.
"""

import numpy as np

import concourse.bass as bass
import concourse.tile as tile
from concourse import mybir
from concourse.bass_utils import run_bass_kernel_spmd

B, H, W, C, HEADS, SR = 4, 64, 64, 256, 8, 2
D = C // HEADS            # 32
N = H * W                 # 4096
M = (H // SR) * (W // SR) # 1024
SCALE = float(D) ** -0.5
EPS = 1e-6
NCORES = 8
PW = 68                   # padded image width/height (64 + 1 left + 3 right)

F32 = mybir.dt.float32
F32R = mybir.dt.float32r

_CACHED = {}


class _SplitDrainTileContext(tile.TileContext):
    """This env's walrus rejects >1 sync wait on TPB_CTRL ops; TileContext's
    tail drain carries one wait per live semaphore.  Split the extras over a
    chain of SP NOPs (program order preserves semantics)."""

    MAX_WAITS = 1

    def _drain_and_barrier(self, tick_clock, wait_clock):
        nc = self.nc
        from concourse.tile import ScopedClock

        drain_inst = nc.sync.drain()
        wait_clock.add_sem_waits(
            drain_inst.ins, ScopedClock({None: tick_clock.global_clock})
        )
        si = drain_inst.ins.sync_info
        waits = list(si.on_wait) if si is not None and si.on_wait else []
        mw = self.MAX_WAITS
        if len(waits) > mw:
            si.on_wait = waits[:mw]
            rest = waits[mw:]
            for i in range(0, len(rest), mw):
                nop = nc.sync.nop()
                nsi = nop.ins.sync_info
                if nsi is None:
                    nop.ins.sync_info = type(si)(
                        on_wait=rest[i : i + mw], on_update=[]
                    )
                else:
                    nsi.on_wait = rest[i : i + mw]

        nc.all_engine_barrier()
        assert self.sems is not None
        popped = nc._tile_sem_poison_stack.pop()
        assert popped is self._sem_poison
        nc.clear_and_free_semaphores(list(self.sems.allocated().values()))
        nc.all_engine_barrier()


def _split_waits(nc):
    """This env's walrus allows only one sync-wait command per instruction
    (CTRL and LDWEIGHTS structs).  Move extra waits onto same-engine NOPs
    spliced immediately before the owning instruction."""
    k = 0
    for bb in nc.m.functions[0].blocks:
        new_insts = []
        for inst in bb.instructions:
            si = inst.sync_info
            waits = list(si.on_wait) if si is not None and si.on_wait else []
            if len(waits) > 1:
                for w in waits[:-1]:
                    nop = mybir.InstNoOp(name=f"wsplit-{k}", ins=[], outs=[])
                    k += 1
                    nop.engine = inst.engine
                    nop.sync_info = mybir.SyncInfo(on_wait=[w], on_update=[])
                    new_insts.append(nop)
                si.on_wait = [waits[-1]]
            new_insts.append(inst)
        bb.instructions[:] = new_insts


def _r(ap):
    return ap if ap.dtype == F32R else ap.bitcast(F32R)


def _build_nc(repeat=1):
    nc = bass.Bass()

    params = {}
    for name, shape in [
        ("xb", [N, C]),
        ("dwcol", [2, 128, 9]),
        ("srcol", [2, 128, 9]),
        ("pwT", [2, 128, 128]),
        ("kvwT", [2, 128, 256]),
        ("projT", [128, 256]),
        ("qbias", [128, 1]),
        ("kvbias", [128, 2]),
        ("lnp", [2, 128, 2]),
        ("onesc", [128, 1]),
        ("ones1", [1, 128]),
        ("band8", [8, 512]),
        ("ident", [128, 128]),
    ]:
        dt = F32R if name in ("pwT", "kvwT", "projT",
                              "onesc", "ones1", "band8", "ident", "xb") else F32
        params[name] = nc.declare_dram_parameter(name, shape, dt, isOutput=False)
    params["y"] = nc.declare_dram_parameter("y", [N, C], F32, isOutput=True)

    with _SplitDrainTileContext(nc) as tc:
        with nc.allow_low_precision(reason="fp32r matmul operands are rounded"):
            if repeat == 1:
                _emit(nc, tc, params)
            else:
                # device-side repeat loop: the program holds ONE body; the
                # loop re-executes it, so repeat-differencing isolates pure
                # HW re-execution time (compile/serialize cost no longer
                # scales with the repeat count).
                with tc.For_i(0, repeat):
                    _emit(nc, tc, params)
    _split_waits(nc)
    return nc


def _emit(nc, tc, t):
    import os
    SKIP = set(os.environ.get("KERNEL_SKIP", "").split(","))
    y = t["y"]
    Exp = mybir.ActivationFunctionType.Exp
    Sqrt = mybir.ActivationFunctionType.Sqrt
    mult = mybir.AluOpType.mult
    add = mybir.AluOpType.add
    subtract = mybir.AluOpType.subtract

    with tc.tile_pool(name="consts", bufs=1) as cpool:
        dwcol_sb = [cpool.tile([128, 9], F32, tag=f"dwc{ct}", name=f"dwc{ct}") for ct in range(2)]
        srcol_sb = [cpool.tile([128, 9], F32, tag=f"src{ct}", name=f"src{ct}") for ct in range(2)]
        pw_sb = [cpool.tile([128, 128], F32R, tag=f"pw{ct}", name=f"pw{ct}") for ct in range(2)]
        kvw_sb = [cpool.tile([128, 256], F32R, tag=f"kvw{ct}", name=f"kvw{ct}") for ct in range(2)]
        projT_sb = cpool.tile([128, 256], F32R, tag="projT", name="projT")
        qbias_sb = cpool.tile([128, 1], F32, tag="qbias", name="qbias")
        kvbias_sb = cpool.tile([128, 2], F32, tag="kvbias", name="kvbias")
        lnp_sb = [cpool.tile([128, 2], F32, tag=f"lnp{ct}", name=f"lnp{ct}") for ct in range(2)]
        onesc_sb = cpool.tile([128, 1], F32R, tag="onesc", name="onesc")
        ones1_sb = cpool.tile([1, 128], F32R, tag="ones1", name="ones1")
        band_sb = cpool.tile([8, 512], F32R, tag="band8", name="band8")
        ident_sb = cpool.tile([128, 128], F32R, tag="ident", name="ident")

        for ct in range(2):
            nc.sync.dma_start(dwcol_sb[ct][:], t["dwcol"][ct])
            nc.sync.dma_start(srcol_sb[ct][:], t["srcol"][ct])
            nc.sync.dma_start(pw_sb[ct][:], t["pwT"][ct])
            nc.sync.dma_start(kvw_sb[ct][:], t["kvwT"][ct])
            nc.sync.dma_start(lnp_sb[ct][:], t["lnp"][ct])
        nc.sync.dma_start(projT_sb[:], t["projT"][:])
        nc.sync.dma_start(qbias_sb[:], t["qbias"][:])
        nc.sync.dma_start(kvbias_sb[:], t["kvbias"][:])
        nc.sync.dma_start(onesc_sb[:], t["onesc"][:])
        nc.sync.dma_start(ones1_sb[:], t["ones1"][:])
        nc.sync.dma_start(band_sb[:], t["band8"][:])
        nc.sync.dma_start(ident_sb[:], t["ident"][:])

        # long-lived intermediates
        with tc.tile_pool(name="live", bufs=1) as lp:
            q_sb = lp.tile([128, N], F32R, tag="q", name="q")
            kvT = [lp.tile([128, M], F32R, tag=f"kvT{jt}", name=f"kvT{jt}") for jt in range(2)]
            v_sb = lp.tile([128, 8, 132], F32R, tag="v", name="v")
            OT = lp.tile([128, N], F32, tag="OT", name="OT")
            sexp = lp.tile([8, 2048], F32, tag="sexp", name="sexp")
            pad = [lp.tile([128, PW, PW], F32R, tag=f"pad{ct}", name=f"pad{ct}")
                   for ct in range(2)]
            dwout = [lp.tile([128, N], F32R, tag=f"dwout{ct}", name=f"dwout{ct}")
                     for ct in range(2)]
            tmp = lp.tile([128, 2048], F32, tag="cvtmp", name="cvtmp")

            _emit_phase1(nc, tc, t, dict(
                q_sb=q_sb, kvT=kvT, pad=pad, tmp=tmp,
                dwcol_sb=dwcol_sb, srcol_sb=srcol_sb,
                pw_sb=pw_sb, kvw_sb=kvw_sb,
                qbias_sb=qbias_sb, kvbias_sb=kvbias_sb, lnp_sb=lnp_sb,
                onesc_sb=onesc_sb, ones1_sb=ones1_sb, ident_sb=ident_sb,
            ))

            # ---- transpose V -> v_sb[:, mt, h*33 .. h*33+32] (+ ones col)
            with tc.tile_pool(name="vtpsum", bufs=4, space="PSUM") as vtp:
                for g in range(2):  # 4 m-tiles per group
                    ps = vtp.tile([128, 512], F32, tag="vt", name="vt")
                    for j in range(4):
                        mt = g * 4 + j
                        nc.tensor.transpose(
                            ps[:, j * 128 : j * 128 + 128].bitcast(F32R),
                            kvT[1][:, mt * 128 : mt * 128 + 128], ident_sb[:]
                        )
                    # ps cols j*128 + h*32 + d  ->  v_sb[:, g*4+j, h*33+d]
                    dst = v_sb[:, g * 4 : g * 4 + 4].rearrange(
                        "p mt (h e) -> p mt h e", e=33
                    )
                    nc.vector.tensor_copy(
                        dst[:, :, :, 0:32],
                        ps[:].rearrange("p (mt h d) -> p mt h d", h=4, d=32),
                    )
                ones_cols = v_sb.rearrange("p mt (h e) -> p mt h e", e=33)
                nc.vector.memset(ones_cols[:, :, :, 32:33].bitcast(F32), 1.0)

            # ---- attention: per (head, n-chunk of 2048)
            with (
                tc.tile_pool(name="spsum", bufs=2, space="PSUM") as sp,
                tc.tile_pool(name="opsum", bufs=2, space="PSUM") as op,
                tc.tile_pool(name="expp", bufs=8) as ep,
                tc.tile_pool(name="sxscr", bufs=2) as sxp,
            ):
                NTAP2 = 1 if "conv" in SKIP else 9

                def emit_dw(half):
                    h0 = half * 32
                    for ct in range(2):
                        dst = dwout[ct][:, half * 2048 : half * 2048 + 2048]
                        dst3 = dst.rearrange("c (hh w) -> c hh w", hh=32)
                        for tap in range(NTAP2):
                            dy, dx = tap // 3, tap % 3
                            rhs = pad[ct][:, dy + h0 : dy + h0 + 32,
                                          dx : dx + 64]
                            w_ap = dwcol_sb[ct][:, tap : tap + 1]
                            if tap == 0:
                                nc.vector.tensor_scalar_mul(dst3, rhs, w_ap)
                            else:
                                nc.vector.tensor_scalar_mul(tmp[:], rhs, w_ap)
                                nc.vector.tensor_tensor(dst, dst, tmp[:],
                                                        op=add)

                def emit_pw(half):
                    for qc in range(2):
                        ps = sp.tile([128, 1024], F32, tag="s", name="s")
                        for sc in range(2):
                            c0 = half * 2048 + qc * 1024 + sc * 512
                            for ct in range(2):
                                nc.tensor.matmul(
                                    ps[:, sc * 512 : sc * 512 + 512],
                                    pw_sb[ct][:],
                                    dwout[ct][:, c0 : c0 + 512],
                                    start=(ct == 0),
                                    stop=(ct == 1),
                                )
                        nc.vector.tensor_scalar_add(
                            q_sb[:, half * 2048 + qc * 1024 :
                                 half * 2048 + qc * 1024 + 1024],
                            ps[:], qbias_sb[:])

                # dw half 1 runs on the DVE underneath attention nch=0
                emit_dw(0)
                emit_pw(0)
                emit_dw(1)
                for nch in range(2):
                    if nch == 1:
                        emit_pw(1)
                    for h in range(4):
                        n0 = nch * 2048
                        o_ps2 = [op.tile([33, 1024], F32, tag="o", name="o")
                                 for _ in range(2)]
                        if "attn" in SKIP:
                            for o_ in o_ps2:
                                nc.vector.memset(o_[:], 1.0)
                        es = []
                        for mt in range(0 if "attn" in SKIP else 8):
                            e = ep.tile([128, 2048], F32R, tag="e", name="e")
                            for sh in range(2):
                                s_ps = sp.tile([128, 1024], F32, tag="s", name="s")
                                for sc in range(2):
                                    c0 = sh * 1024 + sc * 512
                                    nc.tensor.matmul(
                                        s_ps[:, sc * 512 : sc * 512 + 512],
                                        kvT[0][h * 32 : h * 32 + 32,
                                               mt * 128 : mt * 128 + 128],
                                        q_sb[h * 32 : h * 32 + 32,
                                             n0 + c0 : n0 + c0 + 512],
                                        start=True,
                                        stop=True,
                                        tile_position=(h * 32, 0),
                                    )
                                nc.scalar.activation(
                                    e[:, sh * 1024 : sh * 1024 + 1024],
                                    s_ps[:], Exp, scale=SCALE)
                            es.append(e)
                        for mt in range(0 if "attn" in SKIP else 8):
                            for sc in range(4):
                                nc.tensor.matmul(
                                    o_ps2[sc // 2][:, (sc % 2) * 512 :
                                                    (sc % 2) * 512 + 512],
                                    v_sb[:, mt, h * 33 : h * 33 + 33],
                                    es[mt][:, sc * 512 : sc * 512 + 512],
                                    start=(mt == 0),
                                    stop=(mt == 7),
                                )
                        for half in range(2):
                            nc.vector.tensor_copy(
                                OT[h * 32 : h * 32 + 32,
                                   n0 + half * 1024 : n0 + half * 1024 + 1024],
                                o_ps2[half][0:32, :],
                            )
                            scr = sxp.tile([1, 1024], F32, tag="sx", name="sx")
                            nc.vector.tensor_copy(scr[:], o_ps2[half][32:33, :])
                            nc.sync.dma_start(
                                sexp[h * 2 + nch : h * 2 + nch + 1,
                                     half * 1024 : half * 1024 + 1024], scr[:]
                            )

            # ---- normalize O^T and project (partial y for this head-half)
            with (
                tc.tile_pool(name="normp", bufs=1) as np_,
                tc.tile_pool(name="otnp", bufs=3) as otnp,
                tc.tile_pool(name="yp", bufs=3) as ypool,
                tc.tile_pool(name="nbpsum", bufs=2, space="PSUM") as nbp,
                tc.tile_pool(name="ypsum", bufs=1, space="PSUM") as yp,
            ):
                recip = np_.tile([8, 2048], F32R, tag="recip", name="recip")
                nc.vector.reciprocal(recip[:], sexp[:])
                for f0 in range(4):
                    n0 = f0 * 1024
                    rb = nbp.tile([128, 1024], F32, tag="rb", name="rb")
                    for sc in range(2):
                        nc.tensor.matmul(
                            rb[:, sc * 512 : sc * 512 + 512],
                            band_sb[:, f0 * 128 : f0 * 128 + 128],
                            recip[:, (f0 % 2) * 1024 + sc * 512 :
                                  (f0 % 2) * 1024 + sc * 512 + 512],
                            start=True,
                            stop=True,
                        )
                    otn = otnp.tile([128, 1024], F32R, tag="otn", name="otn")
                    nc.vector.tensor_tensor(
                        otn[:], OT[:, n0 : n0 + 1024], rb[:], op=mult
                    )
                    y_ps = yp.tile([128, 2048], F32, tag="ypsm", name="ypsm")
                    for sub in range(8):
                        nc.tensor.matmul(
                            y_ps[:, sub * 256 : sub * 256 + 256],
                            otn[:, sub * 128 : sub * 128 + 128],
                            projT_sb[:],
                            start=True,
                            stop=True,
                        )
                    yt = ypool.tile([128, 2048], F32, tag="yt", name="yt")
                    nc.vector.tensor_copy(yt[:], y_ps[:])
                    nc.sync.dma_start(
                        y.rearrange("(f0 nt p) c -> f0 p nt c", f0=4, p=128)[f0],
                        yt[:].rearrange("p (nt c) -> p nt c", c=256),
                    )


def _emit_phase1(nc, tc, t, s):
    """x load/transpose, dw+pw conv -> q, SR conv -> LN -> kv -> kvT."""
    import os
    q_sb, kvT = s["q_sb"], s["kvT"]
    pw_sb, kvw_sb = s["pw_sb"], s["kvw_sb"]
    ident_sb = s["ident_sb"]
    Sqrt = mybir.ActivationFunctionType.Sqrt
    mult = mybir.AluOpType.mult
    add = mybir.AluOpType.add
    subtract = mybir.AluOpType.subtract

    with (
        tc.tile_pool(name="pA", bufs=1) as pA,
        tc.tile_pool(name="tppsum", bufs=4, space="PSUM") as tpp,
    ):
        pad = s["pad"]
        for ct in range(2):
            nc.vector.memset(pad[ct][:].bitcast(F32), 0.0)
        xview = t["xb"].rearrange("(t p) c -> p t c", p=128)
        with tc.tile_pool(name="xinp", bufs=2) as xp:
            for g in range(0 if "xprep" in os.environ.get("KERNEL_SKIP", "") else 8):
                xin = xp.tile([128, 4, 256], F32R, tag="xin", name="xin")
                nc.sync.dma_start(xin[:], xview[:, g * 4 : g * 4 + 4, :])
                for ct in range(2):
                    ps = tpp.tile([128, 512], F32, tag="tp", name="tp")
                    for j in range(4):
                        nc.tensor.transpose(
                            ps[:, j * 128 : j * 128 + 128].bitcast(F32R),
                            xin[:, j, ct * 128 : ct * 128 + 128],
                            ident_sb[:],
                        )
                    # group g = image rows 8g .. 8g+7
                    nc.vector.tensor_copy(
                        pad[ct][:, 1 + 8 * g : 9 + 8 * g, 1:65], ps[:]
                    )

        tmp = s["tmp"]
        NTAP = 1 if "conv" in os.environ.get("KERNEL_SKIP", "") else 9

        # ---- SR path: strided depthwise on DVE -> xsr [c, 1024]
        xsr = [pA.tile([128, M], F32R, tag=f"xsr{ct}", name=f"xsr{ct}") for ct in range(2)]
        for ct in range(2):
            v5 = pad[ct].rearrange(
                "p (hh h2) (ww w2) -> p hh h2 ww w2", h2=2, w2=2
            )
            for tap in range(NTAP):
                dy, dx = tap // 3, tap % 3
                h0 = dy // 2
                w0 = dx // 2
                rhs = v5[:, h0 : h0 + 32, dy % 2, w0 : w0 + 32, dx % 2]
                w_ap = s["srcol_sb"][ct][:, tap : tap + 1]
                if tap == 0:
                    nc.vector.tensor_scalar_mul(xsr[ct][:], rhs, w_ap)
                else:
                    nc.vector.tensor_scalar_mul(tmp[:, 0:M], rhs, w_ap)
                    nc.vector.tensor_tensor(
                        xsr[ct][:], xsr[ct][:], tmp[:, 0:M], op=add
                    )

        # ---- LayerNorm over c (partitions) via ones-matmuls
        musd = pA.tile([1, 2 * M], F32R, tag="musd", name="musd")  # [mu | inv_sd]
        with tc.tile_pool(name="lnpsum", bufs=1, space="PSUM") as lnpp:
            mean_ps = lnpp.tile([1, M], F32, tag="mean", name="mean")
            msq_ps = lnpp.tile([1, M], F32, tag="msq", name="msq")
            sq = [pA.tile([128, M], F32R, tag=f"sq{ct}", name=f"sq{ct}") for ct in range(2)]
            for ct in range(2):
                nc.vector.tensor_mul(sq[ct][:], xsr[ct][:], xsr[ct][:])
            for ch in range(2):
                for ct in range(2):
                    nc.tensor.matmul(
                        mean_ps[:, ch * 512 : ch * 512 + 512],
                        s["onesc_sb"][:],
                        xsr[ct][:, ch * 512 : ch * 512 + 512],
                        start=(ct == 0),
                        stop=(ct == 1),
                    )
                    nc.tensor.matmul(
                        msq_ps[:, ch * 512 : ch * 512 + 512],
                        s["onesc_sb"][:],
                        sq[ct][:, ch * 512 : ch * 512 + 512],
                        start=(ct == 0),
                        stop=(ct == 1),
                    )
            nc.vector.tensor_copy(musd[:, 0:M], mean_ps[:])
            mu2 = pA.tile([1, M], F32, tag="lnscr", name="mu2")
            nc.vector.tensor_mul(mu2[:], musd[:, 0:M].bitcast(F32), musd[:, 0:M].bitcast(F32))
            nc.vector.tensor_tensor(msq_ps[:], msq_ps[:], mu2[:], op=subtract)
            sd = pA.tile([1, M], F32, tag="lnscr2", name="sd")
            nc.vector.tensor_scalar_add(msq_ps[:], msq_ps[:], EPS)
            nc.scalar.activation(sd[:], msq_ps[:], Sqrt)
            nc.vector.reciprocal(musd[:, M : 2 * M], sd[:])

        # broadcast mu/inv over partitions, apply LN
        xln = [pA.tile([128, M], F32R, tag=f"xln{ct}", name=f"xln{ct}") for ct in range(2)]
        with tc.tile_pool(name="bcpsum", bufs=1, space="PSUM") as bcp:
            mu_b = bcp.tile([128, M], F32, tag="mu_b", name="mu_b")
            inv_b = bcp.tile([128, M], F32, tag="inv_b", name="inv_b")
            for ch in range(4):
                s0 = ch * 512
                nc.tensor.matmul(
                    (mu_b if ch < 2 else inv_b)[:, s0 % M : s0 % M + 512],
                    s["ones1_sb"][:],
                    musd[:, s0 : s0 + 512],
                    start=True,
                    stop=True,
                )
            for ct in range(2):
                lt = tmp[:, 0:M]
                nc.vector.tensor_tensor(
                    lt, xsr[ct][:].bitcast(F32), mu_b[:], op=subtract
                )
                nc.vector.tensor_tensor(lt, lt, inv_b[:], op=mult)
                nc.vector.tensor_scalar(
                    xln[ct][:], lt,
                    s["lnp_sb"][ct][:, 0:1], s["lnp_sb"][ct][:, 1:2],
                    op0=mult, op1=add,
                )

        # ---- kv projection -> kvT[jt] [128, 1024]
        with tc.tile_pool(name="kvpsum", bufs=2, space="PSUM") as kvp:
            for jt in range(2):
                ps = kvp.tile([128, M], F32, tag="kv", name="kv")
                for ch in range(2):
                    for ct in range(2):
                        nc.tensor.matmul(
                            ps[:, ch * 512 : ch * 512 + 512],
                            kvw_sb[ct][:, jt * 128 : jt * 128 + 128],
                            xln[ct][:, ch * 512 : ch * 512 + 512],
                            start=(ct == 0),
                            stop=(ct == 1),
                        )
                nc.vector.tensor_scalar_add(
                    kvT[jt][:], ps[:], s["kvbias_sb"][:, jt : jt + 1],
                )


def _host_prep(pw_w, dw_b, pw_b, dw_w, sr_w, ln_g, ln_b, kv_w, kv_b, proj_w):
    pw2 = pw_w[:, :, 0, 0]                       # [co, ci]

    def cols(tap_w):
        # [2, 128, 9]: per c-tile, per-channel tap weights, tap-major cols
        return np.ascontiguousarray(
            tap_w.reshape(2, 128, 9).astype(np.float32))

    qb_full = pw2 @ dw_b + pw_b                  # [C]
    consts = []
    for hh in range(2):
        co = slice(hh * 128, hh * 128 + 128)
        j_rows = np.r_[hh * 128 : hh * 128 + 128,
                       C + hh * 128 : C + hh * 128 + 128]
        kvsel = kv_w[j_rows, :]                  # [256 j, 256 ci]
        consts.append(dict(
            pwT=np.ascontiguousarray(
                pw2[co, :].T.reshape(2, 128, 128).astype(np.float32)),
            kvwT=np.ascontiguousarray(
                kvsel.T.reshape(2, 128, 256).astype(np.float32)),
            projT=np.ascontiguousarray(
                proj_w[:, co].T.astype(np.float32)),
            qbias=qb_full[co].reshape(128, 1).astype(np.float32),
            kvbias=np.stack([kv_b[j_rows[:128]], kv_b[j_rows[128:]]],
                            axis=1).astype(np.float32),
        ))
    lnp = np.stack(
        [np.stack([ln_g[ct * 128 : ct * 128 + 128],
                   ln_b[ct * 128 : ct * 128 + 128]], axis=1)
         for ct in range(2)]
    ).astype(np.float32)
    # band8[k = h*2 + nc2, f0*128 + p] = 1 iff p//32 == h and nc2 == f0//2
    band8 = np.zeros((8, 512), np.float32)
    for k in range(8):
        h, nc2 = k // 2, k % 2
        for f0 in range(4):
            if f0 // 2 == nc2:
                band8[k, f0 * 128 + h * 32 : f0 * 128 + (h + 1) * 32] = 1.0
    shared = dict(
        dwcol=cols(dw_w[:, 0]),
        srcol=cols(sr_w[:, 0]),
        lnp=lnp,
        onesc=np.full((128, 1), 1.0 / C, np.float32),
        ones1=np.ones((1, 128), np.float32),
        band8=band8,
        ident=np.eye(128, dtype=np.float32),
    )
    return consts, shared


def kernel(x, dw_w, dw_b, pw_w, pw_b, sr_w, ln_g, ln_b, kv_w, kv_b,
           proj_w, proj_b):
    args = [np.asarray(a, np.float32) for a in
            (x, dw_w, dw_b, pw_w, pw_b, sr_w, ln_g, ln_b, kv_w, kv_b,
             proj_w, proj_b)]
    (x, dw_w, dw_b, pw_w, pw_b, sr_w, ln_g, ln_b, kv_w, kv_b,
     proj_w, proj_b) = args

    consts, shared = _host_prep(pw_w, dw_b, pw_b, dw_w, sr_w, ln_g, ln_b,
                                kv_w, kv_b, proj_w)

    import os
    repeat = int(os.environ.get("KERNEL_REPEAT", "1"))
    key = f"nc{repeat}"
    if key not in _CACHED:
        _CACHED[key] = _build_nc(repeat)
    nc = _CACHED[key]

    in_maps = []
    for core in range(NCORES):
        b, hh = core // 2, core % 2
        in_maps.append(dict(xb=np.ascontiguousarray(x[b]),
                            **consts[hh], **shared))

    import os
    kw = {}
    if os.environ.get("KERNEL_TRACE"):
        kw = dict(trace=True)
    rr = run_bass_kernel_spmd(nc, in_maps, list(range(NCORES)), **kw)
    _CACHED["last"] = rr
    res = rr.results
    out = np.empty((B, N, C), np.float32)
    for b in range(B):
        out[b] = res[2 * b]["y"] + res[2 * b + 1]["y"] + proj_b[None, :]
    return out

